# revision 37
# baseline (speedup 1.0000x reference)
"""Trainium2 Bass kernel for nn_DotProductAttention (SQ=SK=2048, B=2, NP=32, HN=64).

Strategy (8 NeuronCores, batch*heads sharded):
  - 64 (b, head) pairs are split 8 per core; each core handles one batch b
    (cores 0-3 -> b=0, cores 4-7 -> b=1), 8 heads, processed as 4 pairs of 2
    heads packed onto the 128 SBUF partitions (h-dim = 64 each).
  - Scores are computed TRANSPOSED: S^T[k, s] = sum_h K[k,h] Q[s,h] via
    matmul(lhsT=K^T chunk [64,128], rhs=Q^T [64,512]) with the two heads of a
    pair row-packed into the 128x128 PE array (tile_position rows 0/64).
  - Softmax without max-subtraction (scores are O(+-8); exp never overflows;
    softmax is shift-invariant so this matches the reference numerically):
      P_unnorm = exp(S/8) * m01,   m01 = 1.0 - mask  (0/1, bf16)
    The mask is applied post-exp as a multiply, which is exactly equivalent to
    the reference's where(mask, -1e4, s) (exp(-1e4 - max) underflows to 0).
  - PV + denominator in one matmul: lhsT = V_aug [128, 65] (col 64 = ones),
    rhs = P^T tiles, accumulated over the 16 k-tiles -> O^T_aug [65, 512]
    where row 64 holds the softmax denominators.
  - O^T_aug is PE-transposed back to [sq, 65]; rows are scaled by the
    reciprocal of col 64 and DMA'd out.
  - Host-side mask-pattern specialization: (sq-block, k-tile) tiles that are
    fully masked (in every batch) are skipped entirely; fully-unmasked tiles
    skip the mask multiply. For a causal mask this removes ~37% of all work
    and most mask multiplies. Correct for arbitrary masks.
  - Matmuls run in float32r (TF32-like, full PE rate); measured end-to-end
    rel err vs the fp32 reference ~4e-4.

The walrus build in this container only accepts ONE sync-wait per
instruction; split_multiwaits() rewrites the Tile-scheduled program to hoist
extra waits onto single-wait NoOps inserted just before the instruction.
"""

import numpy as np

SQ, SK, B, NP, HN = 2048, 2048, 2, 32, 64
NCORES = 8
HPC = B * NP // NCORES          # heads per core = 8
PAIRS = HPC // 2                # 4
P = 128
SQ_BLK = 512
NBLK = SQ // SQ_BLK             # 4
SKT = SK // P                   # 16
VF = HN + 1                     # 65: V columns + ones column (denominator)

_build_cache = {}


def split_multiwaits(nc):
    """Split instructions carrying >1 sem-wait into single-wait NoOp + inst."""
    import concourse.mybir as mybir

    ctr = 0
    for fn in nc.m.functions:
        for bb in fn.blocks:
            out, changed = [], False
            for inst in list(bb.instructions):
                si = inst.sync_info
                waits = list(si.on_wait) if (si is not None and si.on_wait) else []
                if len(waits) > 1:
                    for w in waits[:-1]:
                        ctr += 1
                        out.append(
                            mybir.InstNoOp(
                                name=f"splitwait-{ctr}",
                                engine=inst.engine,
                                sync_info=mybir.SyncInfo(on_wait=[w], on_update=[]),
                            )
                        )
                    si.on_wait = waits[-1:]
                    changed = True
                out.append(inst)
            if changed:
                bb.instructions = out
    return ctr


def _build(active, need_tt, repeat=1, stage="full", cfg=None):
    """Build the Bass program.

    active[j]  : tuple of k-tile indices to process for sq-block j
    need_tt[j] : per k-tile bool, True -> apply the mask multiply
    repeat     : execute the whole workload `repeat` times (timing builds
                 only; output is overwritten identically each time)
    stage      : timing-bisection builds: "full" | "noTT" (skip mask mults)
                 | "qkexp" (QK + exp only) | "loadonly" (DMAs only)
    """
    from contextlib import ExitStack

    import concourse.bass as bass
    import concourse.tile as tile
    from concourse import mybir
    from concourse.masks import make_identity

    f32 = mybir.dt.float32
    f32r = mybir.dt.float32r
    bf16 = mybir.dt.bfloat16
    Exp = mybir.ActivationFunctionType.Exp

    cfg = {**{"ps_bufs": 3, "p_bufs": 8, "qk_bufs": 2, "v_bufs": 2,
              "o_bufs": 8, "mask_slices": True, "ocopy_eng": "vector",
              "defer_out": False, "ov_bufs": 1, "tt_alt": False,
              "mask_pre": False, "tt_first": True, "ov_shared": False,
              "split_loads": False, "interleave": False},
           **(cfg or {})}
    nc = bass.Bass(num_devices=NCORES)
    qT = nc.dram_tensor("qT", [PAIRS, P, SQ], f32r, kind="ExternalInput")
    kT = nc.dram_tensor("kT", [PAIRS, P, SK], f32r, kind="ExternalInput")
    vA = nc.dram_tensor("vA", [HPC, SKT, P, VF], f32r, kind="ExternalInput")
    m01 = nc.dram_tensor("m01", [P, SKT, SQ], bf16, kind="ExternalInput")
    out = nc.dram_tensor("out", [SQ, HPC * HN], f32, kind="ExternalOutput")

    with tile.TileContext(nc) as tc, ExitStack() as ctx:
        const = ctx.enter_context(tc.tile_pool(name="const", bufs=1))
        qk_pool = ctx.enter_context(tc.tile_pool(name="qk", bufs=cfg["qk_bufs"]))
        v_pool = ctx.enter_context(tc.tile_pool(name="v", bufs=cfg["v_bufs"]))
        p_pool = ctx.enter_context(tc.tile_pool(name="p", bufs=cfg["p_bufs"]))
        o_pool = ctx.enter_context(tc.tile_pool(name="o", bufs=cfg["o_bufs"]))
        # 8 PSUM banks total: ps tag = 3 bufs x 2 banks, oA/oB 1 bank each.
        # The small [P, VF] transpose outputs allocate from the same "ps" tag
        # (slots are sized to the max tile with that tag) to avoid needing a
        # dedicated bank pool.
        ps_qk = ctx.enter_context(
            tc.tile_pool(name="psqk", bufs=cfg["ps_bufs"], space="PSUM"))
        ps_ov = ctx.enter_context(
            tc.tile_pool(name="psov", bufs=cfg["ov_bufs"], space="PSUM"))

        ident = const.tile([P, P], f32)
        make_identity(nc, ident)
        m_sb = const.tile([P, SKT, SQ], bf16)

        def load_pair(pair):
            qT_sb = qk_pool.tile([P, SQ], f32r, tag="qT")
            kT_sb = qk_pool.tile([P, SK], f32r, tag="kT")
            if cfg["split_loads"]:
                # halved transfers: the first QK matmuls (block 0, low k-tiles)
                # only gate on the first halves
                nc.sync.dma_start(kT_sb[:, :SK // 2], kT[pair, :, :SK // 2])
                nc.sync.dma_start(qT_sb[:, :SQ // 2], qT[pair, :, :SQ // 2])
                nc.sync.dma_start(kT_sb[:, SK // 2:], kT[pair, :, SK // 2:])
                nc.sync.dma_start(qT_sb[:, SQ // 2:], qT[pair, :, SQ // 2:])
            else:
                nc.sync.dma_start(qT_sb, qT[pair])
                nc.sync.dma_start(kT_sb, kT[pair])
            vA_sb = v_pool.tile([P, SKT, VF], f32r, tag="vA")
            nc.sync.dma_start(vA_sb, vA[2 * pair].rearrange("t p f -> p t f"))
            vB_sb = v_pool.tile([P, SKT, VF], f32r, tag="vB")
            nc.sync.dma_start(vB_sb, vA[2 * pair + 1].rearrange("t p f -> p t f"))
            return qT_sb, kT_sb, vA_sb, vB_sb

        # pair 0's operands first so compute can start immediately; the mask
        # follows as per-k-tile slices so each mask multiply waits only on
        # its own 512 KB slice, not the whole 8 MB transfer
        tiles0 = load_pair(0)
        if cfg["mask_slices"]:
            for t in range(SKT):
                nc.sync.dma_start(m_sb[:, t, :], m01[:, t, :])
        else:
            nc.sync.dma_start(m_sb, m01[:])

        deferred = []

        NU = SQ_BLK // P  # 4 transpose chunks per block

        def emit_out_stage(pair, j, ops_a, ops_b):
            for hi, ops in ((0, ops_a), (1, ops_b)):
                oT = o_pool.tile([VF, SQ_BLK], f32, tag="oT")
                getattr(nc, cfg["ocopy_eng"]).tensor_copy(oT, ops)
                head = 2 * pair + hi
                # reciprocal of the denominator row once, pre-transpose
                nc.vector.reciprocal(oT[HN:VF, :], oT[HN:VF, :])
                # all 4 chunk transposes land in one psum bank: [P, NU, VF]
                tp_full = ps_qk.tile([P, 2, SQ_BLK], f32, tag="ps", name="tp")
                tp = tp_full[:, 0, :NU * VF].rearrange("p (u f) -> p u f", f=VF)
                for u in range(NU):
                    nc.tensor.transpose(
                        tp[:, u, :], oT[:, u * P:(u + 1) * P], ident[0:VF, 0:VF]
                    )
                # single normalize multiply + single store per (head, block)
                # (walrus: only one non-scalar PSUM input per instruction, so
                # stage the reciprocal column through SBUF)
                rv_sb = o_pool.tile([P, NU, 1], f32, tag="rv")
                nc.vector.tensor_copy(rv_sb, tp[:, :, HN:VF])
                o_sb = o_pool.tile([P, NU, HN], f32, tag="osb")
                nc.vector.tensor_mul(
                    o_sb, tp[:, :, 0:HN],
                    rv_sb.to_broadcast([P, NU, HN]),
                )
                nc.sync.dma_start(
                    out[j * SQ_BLK:(j + 1) * SQ_BLK, head * HN:(head + 1) * HN]
                    .rearrange("(u p) f -> p u f", p=P),
                    o_sb,
                )

        if cfg["interleave"]:
            sched = []
            for g in range(0, PAIRS, 2):
                for j in range(NBLK):
                    sched.append((g, j, g == 0 and j == 0))
                    sched.append((g + 1, j, False))
            pair_tiles = {}
        else:
            sched = None

        for pair_rep in range(PAIRS * repeat) if sched is None else range(len(sched)):
            if sched is not None:
                pair, j_only, use0 = sched[pair_rep]
                if pair not in pair_tiles or (j_only == 0 and pair_rep >= 2 * NBLK and pair_tiles.get("gen") != pair // 2):
                    pass
                if pair not in pair_tiles:
                    pair_tiles[pair] = tiles0 if pair == 0 else load_pair(pair)
                qT_sb, kT_sb, vA_sb, vB_sb = pair_tiles[pair]
            else:
                pair = pair_rep % PAIRS
                if pair_rep == 0:
                    qT_sb, kT_sb, vA_sb, vB_sb = tiles0
                else:
                    qT_sb, kT_sb, vA_sb, vB_sb = load_pair(pair)

            if stage == "loadonly":
                continue
            for j in ([j_only] if sched is not None else range(NBLK)):
                s_sl = slice(j * SQ_BLK, (j + 1) * SQ_BLK)
                acts = active[j]
                if cfg["tt_first"]:
                    # masked (diagonal) tiles first so the accumulation tail
                    # of each block is a mask-free tile with a shorter chain
                    acts = tuple(sorted(acts, key=lambda t: not need_tt[j][t]))
                if cfg["ov_shared"]:
                    # one shared 2-slot tag: next block's accumulators can
                    # claim whichever slot drains first
                    ops_a = ps_ov.tile([VF, SQ_BLK], f32, tag="ov", name="oA")
                    ops_b = ps_ov.tile([VF, SQ_BLK], f32, tag="ov", name="oB")
                else:
                    ops_a = ps_ov.tile([VF, SQ_BLK], f32, tag="oA")
                    ops_b = ps_ov.tile([VF, SQ_BLK], f32, tag="oB")
                for idx, t in enumerate(acts):
                    k_sl = slice(t * P, (t + 1) * P)
                    # both heads' S^T tiles in one 2-bank psum tensor so the
                    # exp covers 1024 elements per ACT instruction
                    ps = ps_qk.tile([P, 2, SQ_BLK], f32, tag="ps")
                    nc.tensor.matmul(
                        ps[:, 0, :], lhsT=kT_sb[0:64, k_sl], rhs=qT_sb[0:64, s_sl],
                        start=True, stop=True,
                    )
                    nc.tensor.matmul(
                        ps[:, 1, :], lhsT=kT_sb[64:128, k_sl], rhs=qT_sb[64:128, s_sl],
                        start=True, stop=True,
                    )
                    do_tt = need_tt[j][t] and stage == "full"
                    if do_tt and cfg["mask_pre"]:
                        # m01 holds -30000*mask: add to raw scores in PSUM so
                        # exp underflows to 0 exactly; PV then consumes the
                        # ACT output directly (shorter chain)
                        nc.vector.tensor_add(
                            ps, ps,
                            m_sb[:, t, None, s_sl].to_broadcast([P, 2, SQ_BLK]),
                        )
                    pp = p_pool.tile([P, 2, SQ_BLK], f32r, tag="pp")
                    nc.scalar.activation(pp, ps, Exp, scale=0.125)
                    if do_tt and not cfg["mask_pre"]:
                        tt_eng = (nc.gpsimd if cfg["tt_alt"] and idx % 2
                                  else nc.vector)
                        tt_eng.tensor_mul(
                            pp, pp,
                            m_sb[:, t, None, s_sl].to_broadcast([P, 2, SQ_BLK]),
                        )
                    if stage == "qkexp":
                        continue
                    st, sp = idx == 0, idx == len(acts) - 1
                    nc.tensor.matmul(
                        ops_a, lhsT=vA_sb[:, t, :], rhs=pp[:, 0, :],
                        start=st, stop=sp,
                    )
                    nc.tensor.matmul(
                        ops_b, lhsT=vB_sb[:, t, :], rhs=pp[:, 1, :],
                        start=st, stop=sp,
                    )
                if stage == "qkexp":
                    continue
                if cfg["defer_out"]:
                    deferred.append((pair, j, ops_a, ops_b))
                    if len(deferred) > 1:
                        emit_out_stage(*deferred.pop(0))
                else:
                    emit_out_stage(pair, j, ops_a, ops_b)
        for args in deferred:
            emit_out_stage(*args)

    split_multiwaits(nc)
    return nc


def _mask_flags(mask):
    """Compute per-(sq-block, k-tile) skip / mask-multiply flags.

    mask: [B, SQ, SK] bool (True = masked). Flags are shared by all cores
    (one program), so a tile is skipped only if fully masked in EVERY batch,
    and the multiply is skipped only if fully unmasked in EVERY batch.
    """
    blk = mask.reshape(B, NBLK, SQ_BLK, SKT, P)
    all_masked = blk.all(axis=(2, 4)).all(axis=0)     # [NBLK, SKT]
    any_masked = blk.any(axis=(2, 4)).any(axis=0)     # [NBLK, SKT]
    active = []
    need_tt = []
    for j in range(NBLK):
        acts = tuple(t for t in range(SKT) if not all_masked[j, t])
        if not acts:  # fully-masked row block: fall back to no skipping
            acts = tuple(range(SKT))
        active.append(acts)
        need_tt.append(tuple(bool(any_masked[j, t]) for t in range(SKT)))
    return tuple(active), tuple(map(tuple, need_tt))


def _prepare(query, key, value, attention_mask):
    import ml_dtypes

    query = np.asarray(query, dtype=np.float32)
    key = np.asarray(key, dtype=np.float32)
    value = np.asarray(value, dtype=np.float32)
    mask = np.asarray(attention_mask).astype(bool)[:, 0]   # [B, SQ, SK]

    active, need_tt = _mask_flags(mask)
    cache_key = (active, need_tt)
    if cache_key not in _build_cache:
        _build_cache[cache_key] = _build(active, need_tt)
    nc = _build_cache[cache_key]

    in_maps = []
    for c in range(NCORES):
        b = c // (NCORES // B)
        np_lo = (c % (NCORES // B)) * HPC
        q_c = query[:, b, np_lo:np_lo + HPC, :]          # [SQ, 8, 64]
        k_c = key[:, b, np_lo:np_lo + HPC, :]
        v_c = value[:, b, np_lo:np_lo + HPC, :]
        qT_np = np.ascontiguousarray(q_c.transpose(1, 2, 0)).reshape(PAIRS, P, SQ)
        kT_np = np.ascontiguousarray(k_c.transpose(1, 2, 0)).reshape(PAIRS, P, SK)
        vA_np = np.empty((HPC, SKT, P, VF), np.float32)
        vA_np[:, :, :, :HN] = v_c.transpose(1, 0, 2).reshape(HPC, SKT, P, HN)
        vA_np[:, :, :, HN] = 1.0
        m01_np = np.ascontiguousarray(
            (~mask[b]).T.reshape(SKT, P, SQ).transpose(1, 0, 2)
        ).astype(ml_dtypes.bfloat16)
        in_maps.append({"qT": qT_np, "kT": kT_np, "vA": vA_np, "m01": m01_np})
    return nc, in_maps


def _assemble(results):
    full = np.empty((SQ, B, NP * HN), np.float32)
    for c in range(NCORES):
        b = c // (NCORES // B)
        np_lo = (c % (NCORES // B)) * HPC
        full[:, b, np_lo * HN:(np_lo + HPC) * HN] = results[c]["out"]
    return full


def _ensure_device_backend():
    """run_bass_via_pjrt uses the default-platform jax.devices(); if the
    default is cpu (e.g. a harness pinned it for the reference), switch the
    default to whichever backend exposes the NeuronCores."""
    from concourse._compat import axon_active

    if not axon_active():
        return  # native NRT path; jax not involved
    import jax

    try:
        if len(jax.devices()) >= NCORES and jax.devices()[0].platform != "cpu":
            return
    except Exception:
        pass
    try:
        import jax.extend.backend as jeb

        jax.config.update("jax_platform_name", "")
        jeb.clear_backends()
        jax.devices()
    except Exception:
        pass


def kernel(query, key, value, attention_mask):
    from concourse.bass_utils import run_bass_kernel_spmd

    nc, in_maps = _prepare(query, key, value, attention_mask)
    _ensure_device_backend()
    res = run_bass_kernel_spmd(nc, in_maps, core_ids=list(range(NCORES)))
    return _assemble(res.results)
